# revision 31
# baseline (speedup 1.0000x reference)
"""Trainium2 Bass kernel for the attention-scoring MLP (nn_Attn):

    enc = encoder_outputs.transpose(1,0,2)          # [B,S,Hin]
    a1  = tanh(enc @ W1_enc.T + hidden @ W1_hid.T + b1)
    s   = a1 @ W2[0] (+ b2 -- dropped: softmax shift-invariant)
    s   = where(mask, -inf, s)
    out = softmax(s, axis=-1)[:, None, :]           # [B,1,S]

Sharding: data-parallel over batch B=32 across 8 NeuronCores (4 rows
each), weights replicated, no collectives. Per core the main matmul is
computed transposed -- a1T[h, s] = W1_encT.T @ encT per batch row -- so
the (b1 + hidden@W1_hid.T) term rides the ScalarEngine's per-partition
bias port of the tanh activation, and the W2 contraction is a
PSUM-accumulated M=1 matmul over h-tiles. Matmuls run in bf16 (inputs
pre-transposed and converted host-side so all DMAs are contiguous
row-major loads); accumulation is fp32 in PSUM.
"""

import numpy as np
import ml_dtypes

import concourse.bass as bass
import concourse.tile as tile
from concourse import bacc, mybir
from concourse.bass import ds, ts
from concourse.bass_utils import run_bass_kernel_spmd
from concourse.masks import make_identity

N_CORES = 8
B, S, HIN, H = 32, 1024, 1024, 1024
BL = B // N_CORES          # local batch rows per core
P = 128                    # partitions
IT = HIN // P              # contraction tiles
HT = H // P                # output-feature tiles
NT = 512                   # moving-dim tile (s columns per matmul)
SH = S // NT               # s tiles per batch row
F32 = mybir.dt.float32
BF16 = mybir.dt.bfloat16
AF = mybir.ActivationFunctionType
BF = ml_dtypes.bfloat16

_cached_nc = None
LAST_RESULT = None  # BassKernelResults of the most recent run (for test harness)


def _build():
    global _cached_nc
    if _cached_nc is not None:
        return _cached_nc

    nc = bacc.Bacc("TRN2", target_bir_lowering=False, debug=False,
                   num_devices=N_CORES)

    # encT per batch row: [b, i, s]
    enc_ext = nc.dram_tensor("enc", [BL, HIN, S], BF16, kind="ExternalInput").ap()
    # hiddenT: [i, b]
    hidt_ext = nc.dram_tensor("hiddent", [H, BL], BF16, kind="ExternalInput").ap()
    mneg_ext = nc.dram_tensor("maskneg", [BL * S], F32, kind="ExternalInput").ap()
    # W1 split + transposed: [i, h]
    w1e_ext = nc.dram_tensor("w1e", [HIN, H], BF16, kind="ExternalInput").ap()
    w1h_ext = nc.dram_tensor("w1h", [H, H], BF16, kind="ExternalInput").ap()
    b1_ext = nc.dram_tensor("b1", [H], F32, kind="ExternalInput").ap()
    w2_ext = nc.dram_tensor("w2", [H], BF16, kind="ExternalInput").ap()
    out_ext = nc.dram_tensor("out", [BL, S], F32, kind="ExternalOutput").ap()

    with tile.TileContext(nc) as tc:
        with (
            tc.tile_pool(name="consts", bufs=1) as consts,
            tc.tile_pool(name="encp", bufs=3) as encp,
            tc.tile_pool(name="thp", bufs=7) as thp,
            tc.tile_pool(name="pap", bufs=2, space="PSUM") as pap,
            tc.tile_pool(name="pscp", bufs=2, space="PSUM") as pscp,
            tc.tile_pool(name="psA", bufs=1, space="PSUM") as psA,
            tc.tile_pool(name="psT", bufs=2, space="PSUM") as psTp,
        ):
            # ---- PE warmup: ~4us of junk matmuls with no DMA deps so the
            # HAM clock-gate is already at 8/8 when the real matmuls arrive.
            warm_sb = consts.tile([P, NT], BF16)
            nc.gpsimd.memset(warm_sb[:], 0.0)
            warm_ps = pap.tile([P, NT], F32, tag="pa1")
            for _ in range(30):
                nc.tensor.matmul(warm_ps[:], warm_sb[:, 0:P], warm_sb[:],
                                 start=True, stop=True)

            # ---- resident weights/constants ----
            # DMA emission order = ring service order: first-needed first.
            # w1e_sb[p, it*H + h] = W1[h, it*128+p]  == w1e_ext[it*128+p, h]
            # One DMA per h-tile: the ht=0 matmul group only waits for 256KB
            # of weights instead of the whole 2MB.
            w1e_sb = consts.tile([P, IT * H], BF16)
            for half in range(2):
                for it in range(IT):
                    nc.sync.dma_start(
                        w1e_sb[:, ds(it * H + half * 512, 512)],
                        w1e_ext[ds(it * P, P), ds(half * 512, 512)],
                    )
            hT_sb = consts.tile([P, IT * BL], BF16)
            for it in range(IT):
                nc.sync.dma_start(hT_sb[:, ts(it, BL)], hidt_ext[ds(it * P, P), :])
            # first enc block is prefetched here, before w1h (phase A can wait)
            enc0_sb = encp.tile([P, IT * NT], BF16, tag="enc")
            for it in range(IT):
                nc.scalar.dma_start(enc0_sb[:, ts(it, NT)],
                                    enc_ext[0, ds(it * P, P), ds(0, NT)])
            w1h_sb = consts.tile([P, IT * H], BF16)
            for it in range(IT):
                nc.scalar.dma_start(w1h_sb[:, ds(it * H, H)], w1h_ext[ds(it * P, P), :])
            b1T_sb = consts.tile([P, HT], F32)
            nc.sync.dma_start(b1T_sb[:], b1_ext.rearrange("(ht p) -> p ht", p=P))
            w2T_sb = consts.tile([P, HT], BF16)
            nc.sync.dma_start(w2T_sb[:], w2_ext.rearrange("(ht p) -> p ht", p=P))
            mneg_sb = consts.tile([1, BL * S], F32)
            nc.sync.dma_start(mneg_sb[:], mneg_ext[:])
            ident_sb = consts.tile([BL, BL], F32)
            make_identity(nc, ident_sb[:])

            bias_sb = consts.tile([P, HT * BL], F32)   # [p, ht*BL+b]
            hterm_sb = consts.tile([BL, H], F32)
            scores_sb = consts.tile([1, BL * S], F32)
            c40 = consts.tile([1, 1], F32)
            nc.gpsimd.memset(c40[:], -40.0)
            exps = consts.tile([1, BL * S], F32)
            ssum = consts.tile([1, BL * SH], F32)
            rcp = consts.tile([1, BL], F32)
            attn = consts.tile([1, BL * S], F32)

            # ---- phase A: h_term[b,h] = hidden @ W1_hid.T; bias = h_termT + b1T
            pht = psA.tile([BL, H], F32)
            for it in range(IT):
                lhs = hT_sb[:, ts(it, BL)]
                nc.tensor.matmul(pht[:, 0:NT], lhs,
                                 w1h_sb[:, ds(it * H, NT)],
                                 start=(it == 0), stop=(it == IT - 1))
                nc.tensor.matmul(pht[:, NT:H], lhs,
                                 w1h_sb[:, ds(it * H + NT, NT)],
                                 start=(it == 0), stop=(it == IT - 1))
            nc.scalar.copy(hterm_sb[:], pht[:])
            for ht in range(HT):
                ptT = psTp.tile([P, BL], F32)
                nc.tensor.transpose(ptT[:], hterm_sb[:, ts(ht, P)], ident_sb[:])
                nc.vector.tensor_scalar_add(bias_sb[:, ts(ht, BL)], ptT[:],
                                            b1T_sb[:, ds(ht, 1)])

            # ---- phase B: per (b, s-half) tile of 512 sequence positions
            for t in range(BL * SH):
                b, sh = divmod(t, SH)
                # encT block: enc_sb[p, it*NT + s] = enc_ext[b, it*128+p, sh*NT+s]
                if t == 0:
                    enc_sb = enc0_sb
                else:
                    enc_sb = encp.tile([P, IT * NT], BF16, tag="enc")
                    # t==1 rides the scalar ring (startup overlap with w1e on
                    # sync); steady-state tiles use the otherwise-idle sync
                    # ring so DMA triggers never serialize against tanh on ACT.
                    eng = nc.scalar if t == 1 else nc.sync
                    for it in range(IT):
                        eng.dma_start(
                            enc_sb[:, ts(it, NT)],
                            enc_ext[b, ds(it * P, P), ds(sh * NT, NT)],
                        )
                psc = pscp.tile([1, NT], F32)
                # Delay the M=1 scores matmuls so a late bias (phase A is
                # still streaming during t=0) never stalls the in-order PE.
                delay = 4 if t == 0 else 3
                pending = []
                for ht in range(HT):
                    pa1 = pap.tile([P, NT], F32, tag="pa1")
                    for it in range(IT):
                        nc.tensor.matmul(
                            pa1[:],
                            w1e_sb[:, ds(it * H + ht * P, P)],
                            enc_sb[:, ts(it, NT)],
                            start=(it == 0), stop=(it == IT - 1),
                        )
                    th = thp.tile([P, NT], BF16)
                    nc.scalar.activation(th[:], pa1[:], AF.Tanh,
                                         bias=bias_sb[:, ds(ht * BL + b, 1)],
                                         scale=1.0)
                    pending.append((th, ht))
                    if len(pending) > delay:
                        pth, pht_idx = pending.pop(0)
                        nc.tensor.matmul(psc[:], w2T_sb[:, ds(pht_idx, 1)],
                                         pth[:],
                                         start=(pht_idx == 0),
                                         stop=(pht_idx == HT - 1))
                for pth, pht_idx in pending:
                    nc.tensor.matmul(psc[:], w2T_sb[:, ds(pht_idx, 1)],
                                     pth[:], start=(pht_idx == 0),
                                     stop=(pht_idx == HT - 1))
                # scores += mask * -1e30   (scores_sb[0, t*NT:] == scores[b, sh*NT:])
                nc.vector.tensor_add(scores_sb[0:1, ds(t * NT, NT)], psc[:],
                                     mneg_sb[0:1, ds(t * NT, NT)])

                # ---- softmax, pipelined per s-half tile.
                # |scores| <= ||W2||_1 <= 32, so exp(s - 40) never overflows
                # and softmax is shift-invariant -- no max-reduce needed.
                nc.scalar.activation(exps[0:1, ds(t * NT, NT)],
                                     scores_sb[0:1, ds(t * NT, NT)],
                                     AF.Exp, bias=c40[0:1, 0:1], scale=1.0,
                                     accum_out=ssum[0:1, ds(t, 1)])
                if sh == SH - 1:
                    # total = sum of the SH per-tile partial sums for row b
                    nc.vector.reduce_sum(rcp[0:1, ds(b, 1)],
                                         ssum[0:1, ds(b * SH, SH)],
                                         axis=mybir.AxisListType.X)
                    nc.vector.reciprocal(rcp[0:1, ds(b, 1)], rcp[0:1, ds(b, 1)])
                    nc.vector.tensor_scalar_mul(attn[0:1, ds(b * S, S)],
                                                exps[0:1, ds(b * S, S)],
                                                rcp[0:1, ds(b, 1)])
                    nc.sync.dma_start(out_ext[b, :], attn[0:1, ds(b * S, S)])

    nc.compile()
    _cached_nc = nc
    return nc


def kernel(hidden, encoder_outputs, mask, W1, b1, W2, b2):
    global LAST_RESULT
    nc = _build()

    enc = np.asarray(encoder_outputs, dtype=np.float32)
    # [S,B,Hin] -> [B,Hin,S] in bf16 so per-core DMAs are contiguous
    enc_t = np.ascontiguousarray(np.transpose(enc, (1, 2, 0)).astype(BF))
    hid_t = np.ascontiguousarray(np.asarray(hidden, dtype=np.float32).T.astype(BF))  # [H,B]
    maskneg = np.where(np.asarray(mask, dtype=bool), np.float32(-1e30),
                       np.float32(0.0)).astype(np.float32)
    W1 = np.asarray(W1, dtype=np.float32)
    w1e = np.ascontiguousarray(W1[:, :HIN].T.astype(BF))   # [Hin, H]
    w1h = np.ascontiguousarray(W1[:, HIN:].T.astype(BF))   # [H, H]
    b1 = np.ascontiguousarray(np.asarray(b1, dtype=np.float32).reshape(H))
    w2 = np.ascontiguousarray(np.asarray(W2, dtype=np.float32).reshape(H).astype(BF))

    in_maps = []
    for c in range(N_CORES):
        sl = slice(c * BL, (c + 1) * BL)
        in_maps.append({
            "enc": np.ascontiguousarray(enc_t[sl]),
            "hiddent": np.ascontiguousarray(hid_t[:, sl]),
            "maskneg": np.ascontiguousarray(maskneg[sl].reshape(-1)),
            "w1e": w1e,
            "w1h": w1h,
            "b1": b1,
            "w2": w2,
        })

    res = run_bass_kernel_spmd(nc, in_maps, core_ids=list(range(N_CORES)))
    LAST_RESULT = res
    out = np.concatenate([res.results[c]["out"] for c in range(N_CORES)], axis=0)
    return np.ascontiguousarray(out[:, None, :].astype(np.float32))


# revision 32
# speedup vs baseline: 1.0200x; 1.0200x over previous
"""Trainium2 Bass kernel for the attention-scoring MLP (nn_Attn):

    enc = encoder_outputs.transpose(1,0,2)          # [B,S,Hin]
    a1  = tanh(enc @ W1_enc.T + hidden @ W1_hid.T + b1)
    s   = a1 @ W2[0] (+ b2 -- dropped: softmax shift-invariant)
    s   = where(mask, -inf, s)
    out = softmax(s, axis=-1)[:, None, :]           # [B,1,S]

Sharding: data-parallel over batch B=32 across 8 NeuronCores (4 rows
each), weights replicated, no collectives. Per core the main matmul is
computed transposed -- a1T[h, s] = W1_encT.T @ encT per batch row -- so
the (b1 + hidden@W1_hid.T) term rides the ScalarEngine's per-partition
bias port of the tanh activation, and the W2 contraction is a
PSUM-accumulated M=1 matmul over h-tiles. Matmuls run in bf16 (inputs
pre-transposed and converted host-side so all DMAs are contiguous
row-major loads); accumulation is fp32 in PSUM.
"""

import numpy as np
import ml_dtypes

import concourse.bass as bass
import concourse.tile as tile
from concourse import bacc, mybir
from concourse.bass import ds, ts
from concourse.bass_utils import run_bass_kernel_spmd
from concourse.masks import make_identity

N_CORES = 8
B, S, HIN, H = 32, 1024, 1024, 1024
BL = B // N_CORES          # local batch rows per core
P = 128                    # partitions
IT = HIN // P              # contraction tiles
HT = H // P                # output-feature tiles
NT = 512                   # moving-dim tile (s columns per matmul)
SH = S // NT               # s tiles per batch row
F32 = mybir.dt.float32
BF16 = mybir.dt.bfloat16
AF = mybir.ActivationFunctionType
BF = ml_dtypes.bfloat16

_cached_nc = None
LAST_RESULT = None  # BassKernelResults of the most recent run (for test harness)


def _build():
    global _cached_nc
    if _cached_nc is not None:
        return _cached_nc

    nc = bacc.Bacc("TRN2", target_bir_lowering=False, debug=False,
                   num_devices=N_CORES)

    # encT per batch row: [b, i, s]
    enc_ext = nc.dram_tensor("enc", [BL, HIN, S], BF16, kind="ExternalInput").ap()
    # hiddenT: [i, b]
    hidt_ext = nc.dram_tensor("hiddent", [H, BL], BF16, kind="ExternalInput").ap()
    mneg_ext = nc.dram_tensor("maskneg", [BL * S], F32, kind="ExternalInput").ap()
    # W1 split + transposed: [i, h]
    w1e_ext = nc.dram_tensor("w1e", [HIN, H], BF16, kind="ExternalInput").ap()
    w1h_ext = nc.dram_tensor("w1h", [H, H], BF16, kind="ExternalInput").ap()
    b1_ext = nc.dram_tensor("b1", [H], F32, kind="ExternalInput").ap()
    w2_ext = nc.dram_tensor("w2", [H], BF16, kind="ExternalInput").ap()
    out_ext = nc.dram_tensor("out", [BL, S], F32, kind="ExternalOutput").ap()

    with tile.TileContext(nc) as tc:
        with (
            tc.tile_pool(name="consts", bufs=1) as consts,
            tc.tile_pool(name="encp", bufs=3) as encp,
            tc.tile_pool(name="thp", bufs=7) as thp,
            tc.tile_pool(name="pap", bufs=2, space="PSUM") as pap,
            tc.tile_pool(name="pscp", bufs=2, space="PSUM") as pscp,
            tc.tile_pool(name="psA", bufs=1, space="PSUM") as psA,
            tc.tile_pool(name="psT", bufs=2, space="PSUM") as psTp,
        ):
            # ---- PE warmup: ~4us of junk matmuls with no DMA deps so the
            # HAM clock-gate is already at 8/8 when the real matmuls arrive.
            warm_sb = consts.tile([P, NT], BF16)
            nc.gpsimd.memset(warm_sb[:], 0.0)
            warm_ps = pap.tile([P, NT], F32, tag="pa1")
            for _ in range(18):
                nc.tensor.matmul(warm_ps[:], warm_sb[:, 0:P], warm_sb[:],
                                 start=True, stop=True)

            # ---- resident weights/constants ----
            # DMA emission order = ring service order: first-needed first.
            # w1e_sb[p, it*H + h] = W1[h, it*128+p]  == w1e_ext[it*128+p, h]
            # One DMA per h-tile: the ht=0 matmul group only waits for 256KB
            # of weights instead of the whole 2MB.
            w1e_t = []
            for it in range(IT):
                w = consts.tile([P, H], BF16, tag=f"w1e{it}")
                nc.sync.dma_start(w[:], w1e_ext[ds(it * P, P), :])
                w1e_t.append(w)
            hT_sb = consts.tile([P, IT * BL], BF16)
            for it in range(IT):
                nc.sync.dma_start(hT_sb[:, ts(it, BL)], hidt_ext[ds(it * P, P), :])
            # first enc block is prefetched here, before w1h (phase A can wait)
            enc0_sb = encp.tile([P, IT * NT], BF16, tag="enc")
            for it in range(IT):
                nc.scalar.dma_start(enc0_sb[:, ts(it, NT)],
                                    enc_ext[0, ds(it * P, P), ds(0, NT)])
            w1h_t = []
            for it in range(IT):
                w = consts.tile([P, H], BF16, tag=f"w1h{it}")
                nc.scalar.dma_start(w[:], w1h_ext[ds(it * P, P), :])
                w1h_t.append(w)
            b1T_sb = consts.tile([P, HT], F32)
            nc.sync.dma_start(b1T_sb[:], b1_ext.rearrange("(ht p) -> p ht", p=P))
            w2T_sb = consts.tile([P, HT], BF16)
            nc.sync.dma_start(w2T_sb[:], w2_ext.rearrange("(ht p) -> p ht", p=P))
            mneg_sb = consts.tile([1, BL * S], F32)
            nc.sync.dma_start(mneg_sb[:], mneg_ext[:])
            ident_sb = consts.tile([BL, BL], F32)
            make_identity(nc, ident_sb[:])

            bias_sb = consts.tile([P, HT * BL], F32)   # [p, ht*BL+b]
            hterm_sb = consts.tile([BL, H], F32)
            scores_sb = consts.tile([1, BL * S], F32)
            c40 = consts.tile([1, 1], F32)
            nc.gpsimd.memset(c40[:], -40.0)
            exps = consts.tile([1, BL * S], F32)
            ssum = consts.tile([1, BL * SH], F32)
            rcp = consts.tile([1, BL], F32)
            attn = consts.tile([1, BL * S], F32)

            # ---- phase A: h_term[b,h] = hidden @ W1_hid.T; bias = h_termT + b1T
            pht = psA.tile([BL, H], F32)
            for it in range(IT):
                lhs = hT_sb[:, ts(it, BL)]
                nc.tensor.matmul(pht[:, 0:NT], lhs,
                                 w1h_t[it][:, ds(0, NT)],
                                 start=(it == 0), stop=(it == IT - 1))
                nc.tensor.matmul(pht[:, NT:H], lhs,
                                 w1h_t[it][:, ds(NT, NT)],
                                 start=(it == 0), stop=(it == IT - 1))
            nc.scalar.copy(hterm_sb[:], pht[:])
            for ht in range(HT):
                ptT = psTp.tile([P, BL], F32)
                nc.tensor.transpose(ptT[:], hterm_sb[:, ts(ht, P)], ident_sb[:])
                nc.vector.tensor_scalar_add(bias_sb[:, ts(ht, BL)], ptT[:],
                                            b1T_sb[:, ds(ht, 1)])

            # ---- phase B: per (b, s-half) tile of 512 sequence positions
            for t in range(BL * SH):
                b, sh = divmod(t, SH)
                # encT block: enc_sb[p, it*NT + s] = enc_ext[b, it*128+p, sh*NT+s]
                if t == 0:
                    enc_sb = enc0_sb
                else:
                    enc_sb = encp.tile([P, IT * NT], BF16, tag="enc")
                    # t==1 rides the scalar ring (startup overlap with w1e on
                    # sync); steady-state tiles use the otherwise-idle sync
                    # ring so DMA triggers never serialize against tanh on ACT.
                    eng = nc.scalar if t == 1 else nc.sync
                    for it in range(IT):
                        eng.dma_start(
                            enc_sb[:, ts(it, NT)],
                            enc_ext[b, ds(it * P, P), ds(sh * NT, NT)],
                        )
                psc = pscp.tile([1, NT], F32)
                # Delay the M=1 scores matmuls so a late bias (phase A is
                # still streaming during t=0) never stalls the in-order PE.
                delay = 4 if t == 0 else 3
                pending = []
                for ht in range(HT):
                    pa1 = pap.tile([P, NT], F32, tag="pa1")
                    for it in range(IT):
                        nc.tensor.matmul(
                            pa1[:],
                            w1e_t[it][:, ds(ht * P, P)],
                            enc_sb[:, ts(it, NT)],
                            start=(it == 0), stop=(it == IT - 1),
                        )
                    th = thp.tile([P, NT], BF16)
                    nc.scalar.activation(th[:], pa1[:], AF.Tanh,
                                         bias=bias_sb[:, ds(ht * BL + b, 1)],
                                         scale=1.0)
                    pending.append((th, ht))
                    if len(pending) > delay:
                        pth, pht_idx = pending.pop(0)
                        nc.tensor.matmul(psc[:], w2T_sb[:, ds(pht_idx, 1)],
                                         pth[:],
                                         start=(pht_idx == 0),
                                         stop=(pht_idx == HT - 1))
                for pth, pht_idx in pending:
                    nc.tensor.matmul(psc[:], w2T_sb[:, ds(pht_idx, 1)],
                                     pth[:], start=(pht_idx == 0),
                                     stop=(pht_idx == HT - 1))
                # scores += mask * -1e30   (scores_sb[0, t*NT:] == scores[b, sh*NT:])
                nc.vector.tensor_add(scores_sb[0:1, ds(t * NT, NT)], psc[:],
                                     mneg_sb[0:1, ds(t * NT, NT)])

                # ---- softmax, pipelined per s-half tile.
                # |scores| <= ||W2||_1 <= 32, so exp(s - 40) never overflows
                # and softmax is shift-invariant -- no max-reduce needed.
                nc.scalar.activation(exps[0:1, ds(t * NT, NT)],
                                     scores_sb[0:1, ds(t * NT, NT)],
                                     AF.Exp, bias=c40[0:1, 0:1], scale=1.0,
                                     accum_out=ssum[0:1, ds(t, 1)])
                if sh == SH - 1:
                    # total = sum of the SH per-tile partial sums for row b
                    nc.vector.reduce_sum(rcp[0:1, ds(b, 1)],
                                         ssum[0:1, ds(b * SH, SH)],
                                         axis=mybir.AxisListType.X)
                    nc.vector.reciprocal(rcp[0:1, ds(b, 1)], rcp[0:1, ds(b, 1)])
                    nc.vector.tensor_scalar_mul(attn[0:1, ds(b * S, S)],
                                                exps[0:1, ds(b * S, S)],
                                                rcp[0:1, ds(b, 1)])
                    nc.sync.dma_start(out_ext[b, :], attn[0:1, ds(b * S, S)])

    nc.compile()
    _cached_nc = nc
    return nc


def kernel(hidden, encoder_outputs, mask, W1, b1, W2, b2):
    global LAST_RESULT
    nc = _build()

    enc = np.asarray(encoder_outputs, dtype=np.float32)
    # [S,B,Hin] -> [B,Hin,S] in bf16 so per-core DMAs are contiguous
    enc_t = np.ascontiguousarray(np.transpose(enc, (1, 2, 0)).astype(BF))
    hid_t = np.ascontiguousarray(np.asarray(hidden, dtype=np.float32).T.astype(BF))  # [H,B]
    maskneg = np.where(np.asarray(mask, dtype=bool), np.float32(-1e30),
                       np.float32(0.0)).astype(np.float32)
    W1 = np.asarray(W1, dtype=np.float32)
    w1e = np.ascontiguousarray(W1[:, :HIN].T.astype(BF))   # [Hin, H]
    w1h = np.ascontiguousarray(W1[:, HIN:].T.astype(BF))   # [H, H]
    b1 = np.ascontiguousarray(np.asarray(b1, dtype=np.float32).reshape(H))
    w2 = np.ascontiguousarray(np.asarray(W2, dtype=np.float32).reshape(H).astype(BF))

    in_maps = []
    for c in range(N_CORES):
        sl = slice(c * BL, (c + 1) * BL)
        in_maps.append({
            "enc": np.ascontiguousarray(enc_t[sl]),
            "hiddent": np.ascontiguousarray(hid_t[:, sl]),
            "maskneg": np.ascontiguousarray(maskneg[sl].reshape(-1)),
            "w1e": w1e,
            "w1h": w1h,
            "b1": b1,
            "w2": w2,
        })

    res = run_bass_kernel_spmd(nc, in_maps, core_ids=list(range(N_CORES)))
    LAST_RESULT = res
    out = np.concatenate([res.results[c]["out"] for c in range(N_CORES)], axis=0)
    return np.ascontiguousarray(out[:, None, :].astype(np.float32))


# revision 33
# speedup vs baseline: 1.0783x; 1.0572x over previous
"""Trainium2 Bass kernel for the attention-scoring MLP (nn_Attn):

    enc = encoder_outputs.transpose(1,0,2)          # [B,S,Hin]
    a1  = tanh(enc @ W1_enc.T + hidden @ W1_hid.T + b1)
    s   = a1 @ W2[0] (+ b2 -- dropped: softmax shift-invariant)
    s   = where(mask, -inf, s)
    out = softmax(s, axis=-1)[:, None, :]           # [B,1,S]

Sharding: data-parallel over batch B=32 across 8 NeuronCores (4 rows
each), weights replicated, no collectives. Per core the main matmul is
computed transposed -- a1T[h, s] = W1_encT.T @ encT per batch row -- so
the (b1 + hidden@W1_hid.T) term rides the ScalarEngine's per-partition
bias port of the tanh activation, and the W2 contraction is a
PSUM-accumulated M=1 matmul over h-tiles. Matmuls run in bf16 (inputs
pre-transposed and converted host-side so all DMAs are contiguous
row-major loads); accumulation is fp32 in PSUM.
"""

import numpy as np
import ml_dtypes

import concourse.bass as bass
import concourse.tile as tile
from concourse import bacc, mybir
from concourse.bass import ds, ts
from concourse.bass_utils import run_bass_kernel_spmd
from concourse.masks import make_identity

N_CORES = 8
B, S, HIN, H = 32, 1024, 1024, 1024
BL = B // N_CORES          # local batch rows per core
P = 128                    # partitions
IT = HIN // P              # contraction tiles
HT = H // P                # output-feature tiles
NT = 512                   # moving-dim tile (s columns per matmul)
SH = S // NT               # s tiles per batch row
F32 = mybir.dt.float32
BF16 = mybir.dt.bfloat16
AF = mybir.ActivationFunctionType
BF = ml_dtypes.bfloat16

_cached_nc = None
LAST_RESULT = None  # BassKernelResults of the most recent run (for test harness)


def _build():
    global _cached_nc
    if _cached_nc is not None:
        return _cached_nc

    nc = bacc.Bacc("TRN2", target_bir_lowering=False, debug=False,
                   num_devices=N_CORES)

    # encT per batch row: [b, i, s]
    enc_ext = nc.dram_tensor("enc", [BL, HIN, S], BF16, kind="ExternalInput").ap()
    # hiddenT: [i, b]
    hidt_ext = nc.dram_tensor("hiddent", [H, BL], BF16, kind="ExternalInput").ap()
    mneg_ext = nc.dram_tensor("maskneg", [BL * S], F32, kind="ExternalInput").ap()
    # W1 split + transposed: [i, h]
    w1e_ext = nc.dram_tensor("w1e", [HIN, H], BF16, kind="ExternalInput").ap()
    w1h_ext = nc.dram_tensor("w1h", [H, H], BF16, kind="ExternalInput").ap()
    b1_ext = nc.dram_tensor("b1", [H], F32, kind="ExternalInput").ap()
    w2_ext = nc.dram_tensor("w2", [H], BF16, kind="ExternalInput").ap()
    out_ext = nc.dram_tensor("out", [BL, S], F32, kind="ExternalOutput").ap()

    with tile.TileContext(nc) as tc:
        with (
            tc.tile_pool(name="consts", bufs=1) as consts,
            tc.tile_pool(name="encp", bufs=3) as encp,
            tc.tile_pool(name="thp", bufs=7) as thp,
            tc.tile_pool(name="pap", bufs=2, space="PSUM") as pap,
            tc.tile_pool(name="pscp", bufs=2, space="PSUM") as pscp,
            tc.tile_pool(name="psA", bufs=1, space="PSUM") as psA,
            tc.tile_pool(name="psT", bufs=2, space="PSUM") as psTp,
        ):
            # ---- PE warmup: ~4us of junk matmuls with no DMA deps so the
            # HAM clock-gate is already at 8/8 when the real matmuls arrive.
            warm_sb = consts.tile([P, NT], BF16)
            nc.gpsimd.memset(warm_sb[:], 0.0)
            warm_ps = pap.tile([P, NT], F32, tag="pa1")
            for _ in range(18):
                nc.tensor.matmul(warm_ps[:], warm_sb[:, 0:P], warm_sb[:],
                                 start=True, stop=True)

            # ---- resident weights/constants ----
            # DMA emission order = ring service order: first-needed first.
            # w1e_sb[p, it*H + h] = W1[h, it*128+p]  == w1e_ext[it*128+p, h]
            # One DMA per h-tile: the ht=0 matmul group only waits for 256KB
            # of weights instead of the whole 2MB.
            w1e_t = []
            for it in range(IT):
                w = consts.tile([P, H], BF16, tag=f"w1e{it}")
                nc.sync.dma_start(w[:], w1e_ext[ds(it * P, P), :])
                w1e_t.append(w)
            hT_sb = consts.tile([P, IT * BL], BF16)
            for it in range(IT):
                nc.sync.dma_start(hT_sb[:, ts(it, BL)], hidt_ext[ds(it * P, P), :])
            # first enc block is prefetched here, before w1h (phase A can wait)
            enc0_sb = encp.tile([P, IT * NT], BF16, tag="enc")
            for it in range(IT):
                nc.scalar.dma_start(enc0_sb[:, ts(it, NT)],
                                    enc_ext[0, ds(it * P, P), ds(0, NT)])
            w1h_t = []
            for it in range(IT):
                w = consts.tile([P, H], BF16, tag=f"w1h{it}")
                nc.scalar.dma_start(w[:], w1h_ext[ds(it * P, P), :])
                w1h_t.append(w)
            b1T_sb = consts.tile([P, HT], F32)
            nc.sync.dma_start(b1T_sb[:], b1_ext.rearrange("(ht p) -> p ht", p=P))
            w2T_sb = consts.tile([P, HT], BF16)
            nc.sync.dma_start(w2T_sb[:], w2_ext.rearrange("(ht p) -> p ht", p=P))
            mneg_sb = consts.tile([1, BL * S], F32)
            nc.sync.dma_start(mneg_sb[:], mneg_ext[:])
            ident_sb = consts.tile([BL, BL], F32)
            make_identity(nc, ident_sb[:])
            # W2 as a padded [128,128] stationary per h-tile (column 0 = w2
            # chunk, rest zero) so the scores matmul keeps the same PE config
            # as the main matmuls; only row 0 of its PSUM output is used.
            w2pad = consts.tile([P, HT * P], BF16)
            nc.gpsimd.memset(w2pad[:], 0.0)
            for ht in range(HT):
                nc.vector.tensor_copy(w2pad[:, ds(ht * P, 1)], w2T_sb[:, ds(ht, 1)])

            bias_sb = consts.tile([P, HT * BL], F32)   # [p, ht*BL+b]
            hterm_sb = consts.tile([BL, H], F32)
            scores_sb = consts.tile([1, BL * S], F32)
            c40 = consts.tile([1, 1], F32)
            nc.gpsimd.memset(c40[:], -40.0)
            exps = consts.tile([1, BL * S], F32)
            ssum = consts.tile([1, BL * SH], F32)
            rcp = consts.tile([1, BL], F32)
            attn = consts.tile([1, BL * S], F32)

            # ---- phase A: h_term[b,h] = hidden @ W1_hid.T; bias = h_termT + b1T
            pht = psA.tile([BL, H], F32)
            for it in range(IT):
                lhs = hT_sb[:, ts(it, BL)]
                nc.tensor.matmul(pht[:, 0:NT], lhs,
                                 w1h_t[it][:, ds(0, NT)],
                                 start=(it == 0), stop=(it == IT - 1))
                nc.tensor.matmul(pht[:, NT:H], lhs,
                                 w1h_t[it][:, ds(NT, NT)],
                                 start=(it == 0), stop=(it == IT - 1))
            nc.scalar.copy(hterm_sb[:], pht[:])
            for ht in range(HT):
                ptT = psTp.tile([P, BL], F32)
                nc.tensor.transpose(ptT[:], hterm_sb[:, ts(ht, P)], ident_sb[:])
                nc.vector.tensor_scalar_add(bias_sb[:, ts(ht, BL)], ptT[:],
                                            b1T_sb[:, ds(ht, 1)])

            # ---- phase B: per (b, s-half) tile of 512 sequence positions
            for t in range(BL * SH):
                b, sh = divmod(t, SH)
                # encT block: enc_sb[p, it*NT + s] = enc_ext[b, it*128+p, sh*NT+s]
                if t == 0:
                    enc_sb = enc0_sb
                else:
                    enc_sb = encp.tile([P, IT * NT], BF16, tag="enc")
                    # t==1 rides the scalar ring (startup overlap with w1e on
                    # sync); steady-state tiles use the otherwise-idle sync
                    # ring so DMA triggers never serialize against tanh on ACT.
                    eng = nc.scalar if t == 1 else nc.sync
                    for it in range(IT):
                        eng.dma_start(
                            enc_sb[:, ts(it, NT)],
                            enc_ext[b, ds(it * P, P), ds(sh * NT, NT)],
                        )
                psc = pscp.tile([P, NT], F32)
                # Delay the M=1 scores matmuls so a late bias (phase A is
                # still streaming during t=0) never stalls the in-order PE.
                delay = 4 if t == 0 else 3
                pending = []
                for ht in range(HT):
                    pa1 = pap.tile([P, NT], F32, tag="pa1")
                    for it in range(IT):
                        nc.tensor.matmul(
                            pa1[:],
                            w1e_t[it][:, ds(ht * P, P)],
                            enc_sb[:, ts(it, NT)],
                            start=(it == 0), stop=(it == IT - 1),
                        )
                    th = thp.tile([P, NT], BF16)
                    nc.scalar.activation(th[:], pa1[:], AF.Tanh,
                                         bias=bias_sb[:, ds(ht * BL + b, 1)],
                                         scale=1.0)
                    pending.append((th, ht))
                    if len(pending) > delay:
                        pth, pht_idx = pending.pop(0)
                        nc.tensor.matmul(psc[:], w2pad[:, ds(pht_idx * P, P)],
                                         pth[:],
                                         start=(pht_idx == 0),
                                         stop=(pht_idx == HT - 1))
                for pth, pht_idx in pending:
                    nc.tensor.matmul(psc[:], w2pad[:, ds(pht_idx * P, P)],
                                     pth[:], start=(pht_idx == 0),
                                     stop=(pht_idx == HT - 1))
                # scores += mask * -1e30   (scores_sb[0, t*NT:] == scores[b, sh*NT:])
                nc.vector.tensor_add(scores_sb[0:1, ds(t * NT, NT)], psc[0:1, :],
                                     mneg_sb[0:1, ds(t * NT, NT)])

                # ---- softmax, pipelined per s-half tile.
                # |scores| <= ||W2||_1 <= 32, so exp(s - 40) never overflows
                # and softmax is shift-invariant -- no max-reduce needed.
                nc.scalar.activation(exps[0:1, ds(t * NT, NT)],
                                     scores_sb[0:1, ds(t * NT, NT)],
                                     AF.Exp, bias=c40[0:1, 0:1], scale=1.0,
                                     accum_out=ssum[0:1, ds(t, 1)])
                if sh == SH - 1:
                    # total = sum of the SH per-tile partial sums for row b
                    nc.vector.reduce_sum(rcp[0:1, ds(b, 1)],
                                         ssum[0:1, ds(b * SH, SH)],
                                         axis=mybir.AxisListType.X)
                    nc.vector.reciprocal(rcp[0:1, ds(b, 1)], rcp[0:1, ds(b, 1)])
                    nc.vector.tensor_scalar_mul(attn[0:1, ds(b * S, S)],
                                                exps[0:1, ds(b * S, S)],
                                                rcp[0:1, ds(b, 1)])
                    nc.sync.dma_start(out_ext[b, :], attn[0:1, ds(b * S, S)])

    nc.compile()
    _cached_nc = nc
    return nc


def kernel(hidden, encoder_outputs, mask, W1, b1, W2, b2):
    global LAST_RESULT
    nc = _build()

    enc = np.asarray(encoder_outputs, dtype=np.float32)
    # [S,B,Hin] -> [B,Hin,S] in bf16 so per-core DMAs are contiguous
    enc_t = np.ascontiguousarray(np.transpose(enc, (1, 2, 0)).astype(BF))
    hid_t = np.ascontiguousarray(np.asarray(hidden, dtype=np.float32).T.astype(BF))  # [H,B]
    maskneg = np.where(np.asarray(mask, dtype=bool), np.float32(-1e30),
                       np.float32(0.0)).astype(np.float32)
    W1 = np.asarray(W1, dtype=np.float32)
    w1e = np.ascontiguousarray(W1[:, :HIN].T.astype(BF))   # [Hin, H]
    w1h = np.ascontiguousarray(W1[:, HIN:].T.astype(BF))   # [H, H]
    b1 = np.ascontiguousarray(np.asarray(b1, dtype=np.float32).reshape(H))
    w2 = np.ascontiguousarray(np.asarray(W2, dtype=np.float32).reshape(H).astype(BF))

    in_maps = []
    for c in range(N_CORES):
        sl = slice(c * BL, (c + 1) * BL)
        in_maps.append({
            "enc": np.ascontiguousarray(enc_t[sl]),
            "hiddent": np.ascontiguousarray(hid_t[:, sl]),
            "maskneg": np.ascontiguousarray(maskneg[sl].reshape(-1)),
            "w1e": w1e,
            "w1h": w1h,
            "b1": b1,
            "w2": w2,
        })

    res = run_bass_kernel_spmd(nc, in_maps, core_ids=list(range(N_CORES)))
    LAST_RESULT = res
    out = np.concatenate([res.results[c]["out"] for c in range(N_CORES)], axis=0)
    return np.ascontiguousarray(out[:, None, :].astype(np.float32))
